# revision 11
# baseline (speedup 1.0000x reference)
"""GNN message-passing (3x SAGEConv + global mean pool) on 8 TRN2 NeuronCores.

Sharding: nodes in 8 contiguous ranges of 6272 (50000 padded to 50176); each
core owns all edges whose dst lands in its range.  Aggregation per 128-node
dst tile via one-hot matmuls: aggT[f, n] = sum_chunks m_chunk.T @ S_chunk,
where m = gathered relu(h[src]) rows (dma_gather) and S[e, n] =
(segid[e] == n) * inv_cnt[e] built on DVE from iota + per-edge metadata.
dma_gather indices are int16, so the 50176-row table is addressed as two
halves (src < 32768 "lo", src >= 32768 "hi") with separate gather calls.
Between layers the per-core h slices are AllGathered into a replicated
node-major table (bf16) for the next layer's gathers.
"""

import numpy as np
import ml_dtypes

# ---------------------------------------------------------------- constants
N_NODES = 50000
N_EDGES = 600000
D = 128
OUT = 40
NG = 128          # graphs
NCORES = 8
OWN = 6272        # nodes per core (padded)
NPAD = OWN * NCORES   # 50176
TILES = 49        # dst tiles per core (128 nodes each)
TILE_N = 128
HALF = 32768      # int16 index limit for dma_gather
BATCH_T = 7       # dst tiles per gather batch (49 = 7*7)
DOUTS = [128, 128, 40]

TABLE_NP = ml_dtypes.bfloat16   # gather-table / message dtype

import os
DBG_LAYERS = int(os.environ.get("DBG_LAYERS", "3"))
DBG_COLLECTIVE = int(os.environ.get("DBG_COLLECTIVE", "1"))
DBG_GATHER = int(os.environ.get("DBG_GATHER", "1"))
DBG_QUEUES = int(os.environ.get("DBG_QUEUES", "4"))
GATHER_CHUNKS = 8   # max 128-idx chunks per dma_gather call (ring limit 1024 idxs)


# ---------------------------------------------------------------- host side
def _build_schedule(src, dst, inv_cnt):
    """Per-core edge schedule.  Returns (C_LO, C_HI, per-core dict arrays)."""
    order = np.argsort(dst, kind="stable")
    src_s = src[order]
    dst_s = dst[order]
    # global tile boundaries (392 tiles of 128 nodes over [0, NPAD))
    bounds = np.searchsorted(dst_s, np.arange(0, NPAD + 1, TILE_N))

    tiles_lo = []   # [392] arrays of src (lo) per tile
    tiles_hi = []
    tiles_seg_lo = []
    tiles_seg_hi = []
    tiles_scale_lo = []
    tiles_scale_hi = []
    for g in range(NPAD // TILE_N):
        a, b = bounds[g], bounds[g + 1]
        s = src_s[a:b]
        d = dst_s[a:b]
        seg = (d % TILE_N).astype(np.float32)
        scale = inv_cnt[d].astype(np.float32)
        m = s < HALF
        tiles_lo.append(s[m])
        tiles_hi.append(s[~m] - HALF)
        tiles_seg_lo.append(seg[m])
        tiles_seg_hi.append(seg[~m])
        tiles_scale_lo.append(scale[m])
        tiles_scale_hi.append(scale[~m])

    c_lo = max(1, max((len(t) + 127) // 128 for t in tiles_lo))
    c_hi = max(1, max((len(t) + 127) // 128 for t in tiles_hi))

    cores = []
    for c in range(NCORES):
        idx_lo = np.zeros((TILES * c_lo * 128,), np.int16)
        idx_hi = np.zeros((TILES * c_hi * 128,), np.int16)
        meta = np.zeros((TILES * (c_lo + c_hi), 128, 2), np.float32)
        meta[:, :, 0] = 255.0  # pad segid: never matches iota 0..127
        for t in range(TILES):
            g = c * TILES + t
            for half, (cN, idx_arr, srcs, segs, scales) in enumerate([
                (c_lo, idx_lo, tiles_lo[g], tiles_seg_lo[g], tiles_scale_lo[g]),
                (c_hi, idx_hi, tiles_hi[g], tiles_seg_hi[g], tiles_scale_hi[g]),
            ]):
                n = len(srcs)
                base = t * cN * 128
                idx_arr[base:base + n] = srcs.astype(np.int16)
                mbase = t * (c_lo + c_hi) + (c_lo if half else 0)
                for j in range((n + 127) // 128):
                    lo_, hi_ = j * 128, min((j + 1) * 128, n)
                    col = mbase + j
                    meta[col, 0:hi_ - lo_, 0] = segs[lo_:hi_]
                    meta[col, 0:hi_ - lo_, 1] = scales[lo_:hi_]
        # wrap indices for dma_gather: position i = s*16 + p
        wrap = lambda a: np.tile(np.ascontiguousarray(a.reshape(-1, 16).T), (8, 1))
        cores.append({
            "idx_lo": wrap(idx_lo),
            "idx_hi": wrap(idx_hi),
            "meta": np.ascontiguousarray(meta.transpose(1, 0, 2)),  # [128, cols, 2]
        })
    return c_lo, c_hi, cores


def _build_nc(c_lo, c_hi):
    import concourse.bacc as bacc
    import concourse.bass as bass
    import concourse.mybir as mybir
    import concourse.tile as tile

    f32 = mybir.dt.float32
    TDT = mybir.dt.bfloat16
    CPT = c_lo + c_hi

    nc = bacc.Bacc("TRN2", target_bir_lowering=False, debug=False,
                   num_devices=NCORES, num_swdge_queues=DBG_QUEUES)

    # ---- I/O
    r0 = nc.dram_tensor("r0", [NPAD, D], TDT, kind="ExternalInput")
    xT = nc.dram_tensor("xT", [D, OWN], f32, kind="ExternalInput")
    idx_lo_d = nc.dram_tensor("idx_lo", [128, TILES * c_lo * 8], mybir.dt.int16, kind="ExternalInput")
    idx_hi_d = nc.dram_tensor("idx_hi", [128, TILES * c_hi * 8], mybir.dt.int16, kind="ExternalInput")
    meta_d = nc.dram_tensor("meta", [128, TILES * CPT, 2], f32, kind="ExternalInput")
    pmeta_d = nc.dram_tensor("pmeta", [128, TILES, 2], f32, kind="ExternalInput")
    iota_d = nc.dram_tensor("iota", [128, 128], f32, kind="ExternalInput")
    ident_d = nc.dram_tensor("ident", [128, 128], f32, kind="ExternalInput")
    w_d = {}
    for l in range(3):
        w_d[f"wl{l}"] = nc.dram_tensor(f"wl{l}", [D, DOUTS[l]], f32, kind="ExternalInput")
        w_d[f"wr{l}"] = nc.dram_tensor(f"wr{l}", [D, DOUTS[l]], f32, kind="ExternalInput")
        w_d[f"bl{l}"] = nc.dram_tensor(f"bl{l}", [DOUTS[l], 1], f32, kind="ExternalInput")
    out_h = nc.dram_tensor("out_h", [OWN, OUT], f32, kind="ExternalOutput")
    out_g = nc.dram_tensor("out_g", [OUT, NG], f32, kind="ExternalOutput")

    AluOp = mybir.AluOpType
    ActF = mybir.ActivationFunctionType

    with tile.TileContext(nc) as tc:
        with (
            tc.tile_pool(name="cst", bufs=1) as cst,
            tc.tile_pool(name="sb", bufs=1) as sb,
            tc.tile_pool(name="work", bufs=1) as work,
            tc.tile_pool(name="ps", bufs=1, space="PSUM") as ps,
            tc.tile_pool(name="dram", bufs=1, space="DRAM") as dram,
        ):
            # ---- constants to SBUF
            idxlo_sb = cst.tile([128, TILES * c_lo * 8], mybir.dt.int16)
            nc.sync.dma_start(idxlo_sb[:], idx_lo_d[:])
            idxhi_sb = cst.tile([128, TILES * c_hi * 8], mybir.dt.int16)
            nc.sync.dma_start(idxhi_sb[:], idx_hi_d[:])
            meta_sb = cst.tile([128, TILES * CPT, 2], f32)
            nc.sync.dma_start(meta_sb[:], meta_d[:])
            pmeta_sb = cst.tile([128, TILES, 2], f32)
            nc.sync.dma_start(pmeta_sb[:], pmeta_d[:])
            iota_sb = cst.tile([128, 128], f32)
            nc.sync.dma_start(iota_sb[:], iota_d[:])
            ident_sb = cst.tile([128, 128], f32)
            nc.sync.dma_start(ident_sb[:], ident_d[:])
            w_sb = {}
            for l in range(3):
                for k in (f"wl{l}", f"wr{l}"):
                    w_sb[k] = cst.tile([D, DOUTS[l]], f32, name=k)
                    nc.sync.dma_start(w_sb[k][:], w_d[k][:])
                w_sb[f"bl{l}"] = cst.tile([DOUTS[l], 1], f32, name=f"bl{l}")
                nc.sync.dma_start(w_sb[f"bl{l}"][:], w_d[f"bl{l}"][:])

            # ---- h buffers (feature-major, fp32)
            h0T = sb.tile([D, OWN], f32)
            nc.sync.dma_start(h0T[:], xT[:])
            h1T = sb.tile([D, OWN], f32)
            h2T = sb.tile([D, OWN], f32)
            h3T = sb.tile([OUT, OWN], f32)
            hbufs = [h0T, h1T, h2T, h3T]

            # ---- DRAM bounce + tables
            ag_in = [None, None]
            tables = [None, None]
            for i in range(2):
                ag_in[i] = dram.tile([OWN, D], TDT, name=f"ag_in{i}")
                tables[i] = dram.tile([NPAD, D], TDT, name=f"table{i}", addr_space="Shared")

            gT_ps = ps.tile([OUT, NG], f32, tag="gT", bufs=1)

            qrr = [0]  # round-robin SWDGE queue counter
            for l in range(DBG_LAYERS):
                hT = hbufs[l]
                hnT = hbufs[l + 1]
                dout = DOUTS[l]
                if l == 0:
                    tbl_lo, tbl_hi = r0[0:HALF, :], r0[HALF:NPAD, :]
                else:
                    tb = tables[l - 1]
                    tbl_lo, tbl_hi = tb[0:HALF, :], tb[HALF:NPAD, :]

                for b in range(TILES // BATCH_T):
                    ncl = BATCH_T * c_lo
                    nch = BATCH_T * c_hi
                    glo = work.tile([128, ncl, D], TDT, tag="glo", bufs=2)
                    ghi = work.tile([128, nch, D], TDT, tag="ghi", bufs=2)
                    if DBG_GATHER:
                        for gt, tbl, nch_tot, idxs_sb, base_col in (
                            (glo, tbl_lo, ncl, idxlo_sb, b * ncl),
                            (ghi, tbl_hi, nch, idxhi_sb, b * nch),
                        ):
                            for a in range(0, nch_tot, GATHER_CHUNKS):
                                e = min(a + GATHER_CHUNKS, nch_tot)
                                n = (e - a) * 128
                                nc.gpsimd.dma_gather(
                                    gt[:, a:e, :], tbl,
                                    idxs_sb[:, (base_col + a) * 8:(base_col + e) * 8],
                                    n, n, D, elem_step=D,
                                    queue_num=qrr[0] % DBG_QUEUES)
                                qrr[0] += 1
                    else:
                        nc.vector.memset(glo[:], 0.0)
                        nc.vector.memset(ghi[:], 0.0)

                    for tl in range(BATCH_T):
                        t = b * BATCH_T + tl
                        aggT = ps.tile([128, 128], f32, tag="aggT", bufs=2)
                        for j in range(c_lo + c_hi):
                            col = t * CPT + j
                            S = work.tile([128, 128], TDT, tag="S", bufs=4)
                            nc.vector.tensor_scalar(
                                S[:], iota_sb[:],
                                meta_sb[:, col, 0:1], meta_sb[:, col, 1:2],
                                AluOp.is_equal, AluOp.mult)
                            lhs = (glo[:, tl * c_lo + j, :] if j < c_lo
                                   else ghi[:, tl * c_hi + (j - c_lo), :])
                            nc.tensor.matmul(
                                out=aggT[:], lhsT=lhs, rhs=S[:],
                                start=(j == 0), stop=(j == c_lo + c_hi - 1))
                        agg_sb = work.tile([128, 128], f32, tag="agg_sb", bufs=3)
                        nc.scalar.copy(agg_sb[:], aggT[:])
                        lin = ps.tile([dout, 128], f32, tag="lin", bufs=2)
                        nc.tensor.matmul(out=lin[:], lhsT=w_sb[f"wl{l}"][:],
                                         rhs=agg_sb[:], start=True, stop=False)
                        nc.tensor.matmul(out=lin[:], lhsT=w_sb[f"wr{l}"][:],
                                         rhs=hT[:, t * 128:(t + 1) * 128],
                                         start=False, stop=True)
                        if l < 2:
                            nc.scalar.activation(
                                hnT[:, t * 128:(t + 1) * 128], lin[:],
                                ActF.Relu, bias=w_sb[f"bl{l}"][:])
                        else:
                            nc.vector.tensor_scalar_add(
                                hnT[:, t * 128:(t + 1) * 128], lin[:],
                                w_sb[f"bl{l}"][:])

                if l < 2:
                    for t in range(TILES):
                        tp = ps.tile([128, 128], f32, tag="tp", bufs=2)
                        nc.tensor.transpose(tp[:], hnT[:, t * 128:(t + 1) * 128],
                                            ident_sb[:])
                        tpsb = work.tile([128, 128], TDT, tag="tpsb", bufs=3)
                        nc.scalar.copy(tpsb[:], tp[:])
                        nc.sync.dma_start(ag_in[l][t * 128:(t + 1) * 128, :], tpsb[:])
                    if DBG_COLLECTIVE:
                        nc.gpsimd.collective_compute(
                            "AllGather", AluOp.bypass,
                            replica_groups=[list(range(NCORES))],
                            ins=[ag_in[l][:].opt()], outs=[tables[l][:].opt()])
                    else:
                        nc.sync.dma_start(
                            tables[l][0:OWN, :], ag_in[l][0:OWN, :])
                else:
                    for t in range(TILES):
                        tp = ps.tile([128, OUT], f32, tag="tp", bufs=2)
                        nc.tensor.transpose(tp[:], hnT[:, t * 128:(t + 1) * 128],
                                            ident_sb[0:OUT, 0:OUT])
                        tpsb = work.tile([128, OUT], f32, tag="tpsb2", bufs=3)
                        nc.scalar.copy(tpsb[:], tp[:])
                        nc.sync.dma_start(out_h[t * 128:(t + 1) * 128, :], tpsb[:])
                        G = work.tile([128, 128], f32, tag="G", bufs=3)
                        nc.vector.tensor_scalar(
                            G[:], iota_sb[:],
                            pmeta_sb[:, t, 0:1], pmeta_sb[:, t, 1:2],
                            AluOp.is_equal, AluOp.mult)
                        nc.tensor.matmul(out=gT_ps[:], lhsT=tpsb[:], rhs=G[:],
                                         start=(t == 0), stop=(t == TILES - 1),
                                         skip_group_check=True)
                    g_sb = work.tile([OUT, NG], f32)
                    nc.scalar.copy(g_sb[:], gT_ps[:])
                    nc.sync.dma_start(out_g[:], g_sb[:])
    nc.compile()
    return nc


_CACHED = {}


def kernel(x, edge_index, batch, Wl0, bl0, Wr0, Wl1, bl1, Wr1, Wl2, bl2, Wr2):
    from concourse.bass_utils import run_bass_kernel_spmd

    x = np.asarray(x, np.float32)
    ei = np.asarray(edge_index, np.int64)
    batch_np = np.asarray(batch, np.int64)
    src, dst = ei[0], ei[1]

    cnt = np.bincount(dst, minlength=N_NODES).astype(np.float32)
    inv_cnt = (1.0 / np.maximum(cnt, 1.0)).astype(np.float32)
    inv_cnt_pad = np.zeros((NPAD,), np.float32)
    inv_cnt_pad[:N_NODES] = inv_cnt

    c_lo, c_hi, cores = _build_schedule(src, dst, inv_cnt_pad)

    # gather table for layer 0: relu(x), node-major, padded, bf16
    r0 = np.zeros((NPAD, D), TABLE_NP)
    r0[:N_NODES] = np.maximum(x, 0.0).astype(TABLE_NP)

    x_pad = np.zeros((NPAD, D), np.float32)
    x_pad[:N_NODES] = x

    gcnt = np.bincount(batch_np, minlength=NG).astype(np.float32)
    inv_g = (1.0 / np.maximum(gcnt, 1.0)).astype(np.float32)
    batch_pad = np.full((NPAD,), 255.0, np.float32)
    batch_pad[:N_NODES] = batch_np.astype(np.float32)
    invg_pad = np.zeros((NPAD,), np.float32)
    invg_pad[:N_NODES] = inv_g[batch_np]

    iota = np.tile(np.arange(128, dtype=np.float32)[None, :], (128, 1))
    ident = np.eye(128, dtype=np.float32)

    weights = {}
    for l, (Wl, bl, Wr) in enumerate([(Wl0, bl0, Wr0), (Wl1, bl1, Wr1), (Wl2, bl2, Wr2)]):
        weights[f"wl{l}"] = np.ascontiguousarray(np.asarray(Wl, np.float32).T)
        weights[f"wr{l}"] = np.ascontiguousarray(np.asarray(Wr, np.float32).T)
        weights[f"bl{l}"] = np.asarray(bl, np.float32).reshape(-1, 1)

    in_maps = []
    for c in range(NCORES):
        sl = slice(c * OWN, (c + 1) * OWN)
        pm = np.zeros((128, TILES, 2), np.float32)
        bslice = batch_pad[sl].reshape(TILES, 128).T   # [128, TILES]
        gslice = invg_pad[sl].reshape(TILES, 128).T
        pm[:, :, 0] = bslice
        pm[:, :, 1] = gslice
        in_maps.append({
            "r0": r0,
            "xT": np.ascontiguousarray(x_pad[sl].T),
            "idx_lo": cores[c]["idx_lo"],
            "idx_hi": cores[c]["idx_hi"],
            "meta": cores[c]["meta"],
            "pmeta": pm,
            "iota": iota,
            "ident": ident,
            **weights,
        })

    key = (c_lo, c_hi)
    if key not in _CACHED:
        _CACHED[key] = _build_nc(c_lo, c_hi)
    nc = _CACHED[key]

    res = run_bass_kernel_spmd(nc, in_maps, core_ids=list(range(NCORES)),
                               tmpdir=os.environ.get("KERNEL_PROFILE_DIR") or None)
    globals()["_LAST_RES"] = res

    h_full = np.zeros((N_NODES, OUT), np.float32)
    gT = np.zeros((OUT, NG), np.float32)
    for c in range(NCORES):
        a = c * OWN
        b = min((c + 1) * OWN, N_NODES)
        h_full[a:b] = res.results[c]["out_h"][:b - a]
        gT += res.results[c]["out_g"]
    return h_full, np.ascontiguousarray(gT.T)


if __name__ == "__main__":
    import jax
    import reference
    cpu = jax.devices("cpu")[0]
    with jax.default_device(cpu):
        inputs = {k: np.asarray(v) for k, v in reference.setup_inputs().items()}
        eh, eg = reference.reference(**inputs)
        eh, eg = np.asarray(eh), np.asarray(eg)
    import time
    t0 = time.time()
    h, g = kernel(**inputs)
    print(f"kernel wall time: {time.time() - t0:.1f}s")
    def relerr(a, b):
        return np.abs(a - b).max() / (np.abs(b).max() + 1e-12)
    print("h rel err:", relerr(h, eh))
    print("g rel err:", relerr(g, eg))


# revision 15
# speedup vs baseline: 1.0878x; 1.0878x over previous
"""GNN message-passing (3x SAGEConv + global mean pool) on 8 TRN2 NeuronCores.

Sharding: nodes in 8 contiguous ranges of 6272 (50000 padded to 50176); each
core owns all edges whose dst lands in its range.  Aggregation per 128-node
dst tile via one-hot matmuls: aggT[f, n] = sum_chunks m_chunk.T @ S_chunk,
where m = gathered h[src] rows (dma_gather, bf16 table) and S[e, n] =
(segid[e] == n) * inv_cnt[e] precomputed on the host (bf16, streamed from
HBM).  dma_gather indices are int16, so the 50176-row table is addressed as
two halves (src < 32768 "lo", src >= 32768 "hi") with separate gather calls,
each capped at 1024 indices (SWDGE descriptor-ring limit), round-robined
over 4 SWDGE queues.  Between layers the per-core h slices are AllGathered
into a replicated node-major bf16 table for the next layer's gathers.
"""

import numpy as np
import ml_dtypes

# ---------------------------------------------------------------- constants
N_NODES = 50000
N_EDGES = 600000
D = 128
OUT = 40
NG = 128          # graphs
NCORES = 8
OWN = 6272        # nodes per core (padded)
NPAD = OWN * NCORES   # 50176
TILES = 49        # dst tiles per core (128 nodes each)
TILE_N = 128
HALF = 32768      # int16 index limit for dma_gather
BATCH_T = 7       # dst tiles per gather batch (49 = 7*7)
DOUTS = [128, 128, 40]

TABLE_NP = ml_dtypes.bfloat16   # gather-table / message dtype

import os
DBG_LAYERS = int(os.environ.get("DBG_LAYERS", "3"))
DBG_COLLECTIVE = int(os.environ.get("DBG_COLLECTIVE", "1"))
GATHER_CHUNKS = 8   # max 128-idx chunks per dma_gather call (ring limit 1024 idxs)
N_QUEUES = 4


# ---------------------------------------------------------------- host side
def _build_schedule(src, dst, inv_cnt):
    """Per-core edge schedule.  Returns (c_lo, c_hi, per-core dict arrays)."""
    order = np.argsort(dst, kind="stable")
    src_s = src[order]
    dst_s = dst[order]
    bounds = np.searchsorted(dst_s, np.arange(0, NPAD + 1, TILE_N))

    tiles = []  # per global tile: (lo_src, lo_seg, lo_scale, hi_src, hi_seg, hi_scale)
    for g in range(NPAD // TILE_N):
        a, b = bounds[g], bounds[g + 1]
        s = src_s[a:b]
        d = dst_s[a:b]
        seg = (d % TILE_N).astype(np.int32)
        scale = inv_cnt[d].astype(np.float32)
        m = s < HALF
        tiles.append((s[m], seg[m], scale[m], s[~m] - HALF, seg[~m], scale[~m]))

    c_lo = max(1, max((len(t[0]) + 127) // 128 for t in tiles))
    c_hi = max(1, max((len(t[3]) + 127) // 128 for t in tiles))
    cpt = c_lo + c_hi

    cores = []
    for c in range(NCORES):
        idx_lo = np.zeros((TILES * c_lo * 128,), np.int16)
        idx_hi = np.zeros((TILES * c_hi * 128,), np.int16)
        # S[col, e_slot, n] built as [cols, 128, 128] then transposed to [128, cols, 128]
        seg_arr = np.full((TILES * cpt, 128), 255, np.int32)
        scale_arr = np.zeros((TILES * cpt, 128), np.float32)
        for t in range(TILES):
            g = c * TILES + t
            lo_s, lo_g, lo_v, hi_s, hi_g, hi_v = tiles[g]
            for half, (cN, idx_arr, srcs, segs, scales) in enumerate([
                (c_lo, idx_lo, lo_s, lo_g, lo_v),
                (c_hi, idx_hi, hi_s, hi_g, hi_v),
            ]):
                n = len(srcs)
                idx_arr[t * cN * 128: t * cN * 128 + n] = srcs.astype(np.int16)
                mbase = t * cpt + (c_lo if half else 0)
                for j in range((n + 127) // 128):
                    lo_, hi_ = j * 128, min((j + 1) * 128, n)
                    seg_arr[mbase + j, 0:hi_ - lo_] = segs[lo_:hi_]
                    scale_arr[mbase + j, 0:hi_ - lo_] = scales[lo_:hi_]
        S = (seg_arr[:, :, None] == np.arange(128)[None, None, :])
        S = (S * scale_arr[:, :, None]).astype(TABLE_NP)        # [cols, 128, 128]
        S = np.ascontiguousarray(S.transpose(1, 0, 2))           # [128, cols, 128]
        wrap = lambda a: np.tile(np.ascontiguousarray(a.reshape(-1, 16).T), (8, 1))
        cores.append({
            "idx_lo": wrap(idx_lo),
            "idx_hi": wrap(idx_hi),
            "S": S,
        })
    return c_lo, c_hi, cores


def _build_nc(c_lo, c_hi):
    import concourse.bacc as bacc
    import concourse.bass as bass
    import concourse.mybir as mybir
    import concourse.tile as tile

    f32 = mybir.dt.float32
    TDT = mybir.dt.bfloat16
    CPT = c_lo + c_hi

    nc = bacc.Bacc("TRN2", target_bir_lowering=False, debug=False,
                   num_devices=NCORES, num_swdge_queues=N_QUEUES)

    # ---- I/O
    r0 = nc.dram_tensor("r0", [NPAD, D], TDT, kind="ExternalInput")
    xT = nc.dram_tensor("xT", [D, OWN], TDT, kind="ExternalInput")
    idx_lo_d = nc.dram_tensor("idx_lo", [128, TILES * c_lo * 8], mybir.dt.int16, kind="ExternalInput")
    idx_hi_d = nc.dram_tensor("idx_hi", [128, TILES * c_hi * 8], mybir.dt.int16, kind="ExternalInput")
    S_d = nc.dram_tensor("S", [128, TILES * CPT, 128], TDT, kind="ExternalInput")
    G_d = nc.dram_tensor("G", [128, TILES, 128], f32, kind="ExternalInput")
    ident_d = nc.dram_tensor("ident", [128, 128], f32, kind="ExternalInput")
    identb_d = nc.dram_tensor("identb", [128, 128], TDT, kind="ExternalInput")
    w_d = {}
    for l in range(3):
        w_d[f"wl{l}"] = nc.dram_tensor(f"wl{l}", [D, DOUTS[l]], TDT, kind="ExternalInput")
        w_d[f"wr{l}"] = nc.dram_tensor(f"wr{l}", [D, DOUTS[l]], TDT, kind="ExternalInput")
        w_d[f"bl{l}"] = nc.dram_tensor(f"bl{l}", [DOUTS[l], 1], f32, kind="ExternalInput")
    out_h = nc.dram_tensor("out_h", [OWN, OUT], f32, kind="ExternalOutput")
    out_g = nc.dram_tensor("out_g", [OUT, NG], f32, kind="ExternalOutput")

    AluOp = mybir.AluOpType
    ActF = mybir.ActivationFunctionType

    with tile.TileContext(nc) as tc:
        with (
            tc.tile_pool(name="cst", bufs=1) as cst,
            tc.tile_pool(name="sb", bufs=1) as sb,
            tc.tile_pool(name="work", bufs=1) as work,
            tc.tile_pool(name="ps", bufs=1, space="PSUM") as ps,
            tc.tile_pool(name="dram", bufs=1, space="DRAM") as dram,
        ):
            # ---- constants to SBUF
            idxlo_sb = cst.tile([128, TILES * c_lo * 8], mybir.dt.int16)
            nc.sync.dma_start(idxlo_sb[:], idx_lo_d[:])
            idxhi_sb = cst.tile([128, TILES * c_hi * 8], mybir.dt.int16)
            nc.sync.dma_start(idxhi_sb[:], idx_hi_d[:])
            ident_sb = cst.tile([128, 128], f32)
            nc.sync.dma_start(ident_sb[:], ident_d[:])
            identb_sb = cst.tile([128, 128], TDT)
            nc.sync.dma_start(identb_sb[:], identb_d[:])
            w_sb = {}
            for l in range(3):
                for k in (f"wl{l}", f"wr{l}"):
                    w_sb[k] = cst.tile([D, DOUTS[l]], TDT, name=k)
                    nc.sync.dma_start(w_sb[k][:], w_d[k][:])
                w_sb[f"bl{l}"] = cst.tile([DOUTS[l], 1], f32, name=f"bl{l}")
                nc.sync.dma_start(w_sb[f"bl{l}"][:], w_d[f"bl{l}"][:])

            # ---- h buffers (feature-major)
            h0T = sb.tile([D, OWN], TDT)
            nc.sync.dma_start(h0T[:], xT[:])
            h1T = sb.tile([D, OWN], TDT)
            h2T = sb.tile([D, OWN], TDT)
            h3T = sb.tile([OUT, OWN], f32)
            hbufs = [h0T, h1T, h2T, h3T]

            # ---- DRAM bounce + shared table (same addresses every layer)
            ag_in = [dram.tile([OWN, D], TDT, name=f"ag_in{i}") for i in range(2)]
            tables = [dram.tile([NPAD, D], TDT, name=f"table{i}", addr_space="Shared")
                      for i in range(2)]

            gT_ps = ps.tile([OUT, NG], f32, tag="gT", bufs=1)

            qrr = [0]
            for l in range(DBG_LAYERS):
                hT = hbufs[l]
                hnT = hbufs[l + 1]
                dout = DOUTS[l]
                tbl = r0 if l == 0 else tables[l - 1]
                tbl_lo, tbl_hi = tbl[0:HALF, :], tbl[HALF:NPAD, :]

                for b in range(TILES // BATCH_T):
                    ncl = BATCH_T * c_lo
                    nch = BATCH_T * c_hi
                    glo = work.tile([128, ncl, D], TDT, tag="glo", bufs=2)
                    ghi = work.tile([128, nch, D], TDT, tag="ghi", bufs=2)
                    S_sb = work.tile([128, BATCH_T * CPT, 128], TDT, tag="S", bufs=2)
                    nc.sync.dma_start(
                        S_sb[:], S_d[:, b * BATCH_T * CPT:(b + 1) * BATCH_T * CPT, :])
                    for gt, nct, idxs_sb, base_col in (
                        (glo, ncl, idxlo_sb, b * ncl),
                        (ghi, nch, idxhi_sb, b * nch),
                    ):
                        for a in range(0, nct, GATHER_CHUNKS):
                            e = min(a + GATHER_CHUNKS, nct)
                            n = (e - a) * 128
                            nc.gpsimd.dma_gather(
                                gt[:, a:e, :], tbl_lo if gt is glo else tbl_hi,
                                idxs_sb[:, (base_col + a) * 8:(base_col + e) * 8],
                                n, n, D, elem_step=D,
                                queue_num=qrr[0] % N_QUEUES)
                            qrr[0] += 1

                    for tl in range(BATCH_T):
                        t = b * BATCH_T + tl
                        aggT = ps.tile([128, 128], f32, tag="aggT", bufs=2)
                        for j in range(CPT):
                            lhs = (glo[:, tl * c_lo + j, :] if j < c_lo
                                   else ghi[:, tl * c_hi + (j - c_lo), :])
                            nc.tensor.matmul(
                                out=aggT[:], lhsT=lhs,
                                rhs=S_sb[:, tl * CPT + j, :],
                                start=(j == 0), stop=(j == CPT - 1))
                        agg_sb = work.tile([128, 128], TDT, tag="agg_sb", bufs=3)
                        nc.scalar.copy(agg_sb[:], aggT[:])
                        lin = ps.tile([dout, 128], f32, tag="lin", bufs=2)
                        nc.tensor.matmul(out=lin[:], lhsT=w_sb[f"wl{l}"][:],
                                         rhs=agg_sb[:], start=True, stop=False)
                        nc.tensor.matmul(out=lin[:], lhsT=w_sb[f"wr{l}"][:],
                                         rhs=hT[:, t * 128:(t + 1) * 128],
                                         start=False, stop=True)
                        if l < 2:
                            nc.scalar.activation(
                                hnT[:, t * 128:(t + 1) * 128], lin[:],
                                ActF.Relu, bias=w_sb[f"bl{l}"][:])
                        else:
                            nc.vector.tensor_scalar_add(
                                hnT[:, t * 128:(t + 1) * 128], lin[:],
                                w_sb[f"bl{l}"][:])

                if l < 2:
                    for t in range(TILES):
                        tp = ps.tile([128, 128], TDT, tag="tp", bufs=2)
                        nc.tensor.transpose(tp[:], hnT[:, t * 128:(t + 1) * 128],
                                            identb_sb[:])
                        tpsb = work.tile([128, 128], TDT, tag="tpsb", bufs=3)
                        nc.scalar.copy(tpsb[:], tp[:])
                        nc.sync.dma_start(ag_in[l][t * 128:(t + 1) * 128, :], tpsb[:])
                    if DBG_COLLECTIVE:
                        nc.gpsimd.collective_compute(
                            "AllGather", AluOp.bypass,
                            replica_groups=[list(range(NCORES))],
                            ins=[ag_in[l][:].opt()], outs=[tables[l][:].opt()])
                    else:
                        nc.sync.dma_start(tables[l][0:OWN, :], ag_in[l][0:OWN, :])
                else:
                    G_sb = cst.tile([128, TILES, 128], f32)
                    nc.sync.dma_start(G_sb[:], G_d[:])
                    for t in range(TILES):
                        tp = ps.tile([128, OUT], f32, tag="tp", bufs=2)
                        nc.tensor.transpose(tp[:], hnT[:, t * 128:(t + 1) * 128],
                                            ident_sb[0:OUT, 0:OUT])
                        tpsb = work.tile([128, OUT], f32, tag="tpsb2", bufs=3)
                        nc.scalar.copy(tpsb[:], tp[:])
                        nc.sync.dma_start(out_h[t * 128:(t + 1) * 128, :], tpsb[:])
                        nc.tensor.matmul(out=gT_ps[:], lhsT=tpsb[:],
                                         rhs=G_sb[:, t, :],
                                         start=(t == 0), stop=(t == TILES - 1),
                                         skip_group_check=True)
                    g_sb = work.tile([OUT, NG], f32)
                    nc.scalar.copy(g_sb[:], gT_ps[:])
                    nc.sync.dma_start(out_g[:], g_sb[:])
    nc.compile()
    return nc


_CACHED = {}


def kernel(x, edge_index, batch, Wl0, bl0, Wr0, Wl1, bl1, Wr1, Wl2, bl2, Wr2):
    from concourse.bass_utils import run_bass_kernel_spmd

    x = np.asarray(x, np.float32)
    ei = np.asarray(edge_index, np.int64)
    batch_np = np.asarray(batch, np.int64)
    src, dst = ei[0], ei[1]

    cnt = np.bincount(dst, minlength=N_NODES).astype(np.float32)
    inv_cnt = (1.0 / np.maximum(cnt, 1.0)).astype(np.float32)
    inv_cnt_pad = np.zeros((NPAD,), np.float32)
    inv_cnt_pad[:N_NODES] = inv_cnt

    c_lo, c_hi, cores = _build_schedule(src, dst, inv_cnt_pad)

    r0 = np.zeros((NPAD, D), TABLE_NP)
    r0[:N_NODES] = np.maximum(x, 0.0).astype(TABLE_NP)
    x_pad = np.zeros((NPAD, D), np.float32)
    x_pad[:N_NODES] = x

    gcnt = np.bincount(batch_np, minlength=NG).astype(np.float32)
    inv_g = (1.0 / np.maximum(gcnt, 1.0)).astype(np.float32)
    batch_pad = np.full((NPAD,), 255, np.int32)
    batch_pad[:N_NODES] = batch_np
    invg_pad = np.zeros((NPAD,), np.float32)
    invg_pad[:N_NODES] = inv_g[batch_np]

    ident = np.eye(128, dtype=np.float32)

    weights = {}
    for l, (Wl, bl, Wr) in enumerate([(Wl0, bl0, Wr0), (Wl1, bl1, Wr1), (Wl2, bl2, Wr2)]):
        weights[f"wl{l}"] = np.ascontiguousarray(np.asarray(Wl, np.float32).T).astype(TABLE_NP)
        weights[f"wr{l}"] = np.ascontiguousarray(np.asarray(Wr, np.float32).T).astype(TABLE_NP)
        weights[f"bl{l}"] = np.asarray(bl, np.float32).reshape(-1, 1)

    in_maps = []
    for c in range(NCORES):
        sl = slice(c * OWN, (c + 1) * OWN)
        bt = batch_pad[sl].reshape(TILES, 128)          # [TILES, 128 nodes]
        gv = invg_pad[sl].reshape(TILES, 128)
        G = (bt[:, :, None] == np.arange(128)[None, None, :])
        G = (G * gv[:, :, None]).astype(np.float32)      # [TILES, 128, 128]
        G = np.ascontiguousarray(G.transpose(1, 0, 2))   # [128, TILES, 128]
        in_maps.append({
            "r0": r0,
            "xT": np.ascontiguousarray(x_pad[sl].T).astype(TABLE_NP),
            "idx_lo": cores[c]["idx_lo"],
            "idx_hi": cores[c]["idx_hi"],
            "S": cores[c]["S"],
            "G": G,
            "ident": ident,
            "identb": ident.astype(TABLE_NP),
            **weights,
        })

    key = (c_lo, c_hi)
    if key not in _CACHED:
        _CACHED[key] = _build_nc(c_lo, c_hi)
    nc = _CACHED[key]

    res = run_bass_kernel_spmd(nc, in_maps, core_ids=list(range(NCORES)),
                               tmpdir=os.environ.get("KERNEL_PROFILE_DIR") or None)
    globals()["_LAST_RES"] = res

    h_full = np.zeros((N_NODES, OUT), np.float32)
    gT = np.zeros((OUT, NG), np.float32)
    for c in range(NCORES):
        a = c * OWN
        b = min((c + 1) * OWN, N_NODES)
        h_full[a:b] = res.results[c]["out_h"][:b - a]
        gT += res.results[c]["out_g"]
    return h_full, np.ascontiguousarray(gT.T)


if __name__ == "__main__":
    import jax
    import reference
    cpu = jax.devices("cpu")[0]
    with jax.default_device(cpu):
        inputs = {k: np.asarray(v) for k, v in reference.setup_inputs().items()}
        eh, eg = reference.reference(**inputs)
        eh, eg = np.asarray(eh), np.asarray(eg)
    import time
    t0 = time.time()
    h, g = kernel(**inputs)
    print(f"kernel wall time: {time.time() - t0:.1f}s")
    def relerr(a, b):
        return np.abs(a - b).max() / (np.abs(b).max() + 1e-12)
    print("h rel err:", relerr(h, eh))
    print("g rel err:", relerr(g, eg))
